# revision 13
# baseline (speedup 1.0000x reference)
"""Trainium2 Bass kernel for GQA decode attention (nn_Attention_37890201485423).

Tensor-parallel over KV heads: 8 cores x (1 KV head + 5 query heads each).
o_proj is row-sharded with a final AllReduce.

Layout strategy (all host-side prep is numpy; all module math runs on device):
  - K cache shard is shipped pre-transposed per batch as [d=128, t=4096] so the
    scores matmul can use K^T chunks as the stationary operand directly.
  - V cache shard is shipped as 128-token chunks with a ones column appended
    ([t, 129]); the AV matmul then produces numerator and softmax denominator
    in one accumulation group.
  - RoPE is applied on device as a single matmul with a 128x128 block-diagonal
    rotation matrix built from freqs_cis (standard precomputed rotary table).
  - All weights are pre-rearranged partition-major so each DMA is contiguous.

Dtype strategy (PE fp32 matmuls cost 4 cycles/row):
  - Projections and o_proj run as float32r (bit-identical f32 storage, 1
    cycle/row at moving dim >= 256).
  - Attention inner loops (scores, AV) run in bf16: K/V are cast f32->bf16
    during the SWDGE DMA; exp() output is written as bf16. All matmul
    accumulation stays fp32 in PSUM, softmax normalization is fp32.
"""

import sys

import numpy as np

if "/opt/trn_rl_repo" not in sys.path:
    sys.path.insert(0, "/opt/trn_rl_repo")


def _install_ntff_hook():
    """The container's antenv stub lacks axon_hooks; recreate it so
    run_bass_kernel_spmd(trace=True) can capture NTFF profiles via the
    libaxon ctypes path (mirrors trn_agent_boot.trn_boot)."""
    import types

    if "antenv.axon_hooks" in sys.modules:
        return
    mod = types.ModuleType("antenv.axon_hooks")
    mod._hook = None

    def set_axon_ntff_profile_hook(h):
        mod._hook = h

    def get_axon_ntff_profile_hook():
        return mod._hook

    mod.set_axon_ntff_profile_hook = set_axon_ntff_profile_hook
    mod.get_axon_ntff_profile_hook = get_axon_ntff_profile_hook
    sys.modules["antenv.axon_hooks"] = mod
    try:
        import antenv

        antenv.axon_hooks = mod
    except ImportError:
        pass
    try:
        boot_dir = "/root/.axon_site/trn_agent_boot"
        if boot_dir not in sys.path:
            sys.path.insert(0, boot_dir)
        import trn_boot

        hook = trn_boot._ntff_profile_via_ctypes("/opt/axon/libaxon_pjrt.so")
        if hook is not None:
            mod._hook = hook
    except Exception:
        pass


_install_ntff_hook()

DIM, N_HEADS, N_KV, HEAD_DIM = 5120, 40, 8, 128
MAX_BS, MAX_SEQ = 8, 4096
NB = 8  # batch
N_CORES = 8
N_REP = N_HEADS // N_KV  # 5 query heads per kv head
HPC = N_REP  # heads per core
QD = HPC * HEAD_DIM  # 640, per-core q/o width
WKV = QD + 2 * HEAD_DIM  # 896: concat q|k|v projection width per core
KD = 40  # contraction chunks for DIM
NCH = 32  # 4096 / 128 token chunks
VE = HEAD_DIM + 1  # 129, v chunk width with ones column
SCALE = 1.0 / float(np.sqrt(HEAD_DIM))


def _build_rope_matrix(freqs_cis: np.ndarray) -> np.ndarray:
    """lhsT for the rope matmul: out = lhsT.T @ rhs applies the rotation A.

    A[2i,2i]=cos_i, A[2i,2i+1]=-sin_i, A[2i+1,2i]=sin_i, A[2i+1,2i+1]=cos_i
    (matches reference _apply_rope with interleaved even/odd pairs).
    """
    cos = np.asarray(freqs_cis, np.float32)[0, :, 0]
    sin = np.asarray(freqs_cis, np.float32)[0, :, 1]
    A = np.zeros((HEAD_DIM, HEAD_DIM), np.float32)
    idx = np.arange(HEAD_DIM // 2)
    A[2 * idx, 2 * idx] = cos
    A[2 * idx, 2 * idx + 1] = -sin
    A[2 * idx + 1, 2 * idx] = sin
    A[2 * idx + 1, 2 * idx + 1] = cos
    return np.ascontiguousarray(A.T)


def _part_major(w: np.ndarray) -> np.ndarray:
    """[K*128, N] -> [128, K*N] with chunk k in columns k*N:(k+1)*N."""
    k = w.shape[0] // 128
    return np.ascontiguousarray(
        w.reshape(k, 128, w.shape[1]).transpose(1, 0, 2).reshape(128, -1)
    )


def prepare_inputs(x, freqs_cis, cache_k, cache_v, wq, bq, wk, bk, wv, bv, wo, bo):
    """Returns per-core in_maps (list of dicts of numpy arrays)."""
    x = np.asarray(x, np.float32).reshape(NB, DIM)
    arope = _build_rope_matrix(freqs_cis)

    # xt: stationary operand layout for the projections.
    # xt[:, k*8:(k+1)*8][p, c] = x[c, k*128+p]
    xs = x.reshape(NB, KD, 128)
    xt = np.ascontiguousarray(xs.transpose(2, 1, 0).reshape(128, KD * NB))

    bo8 = (np.asarray(bo, np.float32) / N_CORES).reshape(1, DIM)
    wq, wk, wv = (np.asarray(a, np.float32) for a in (wq, wk, wv))
    bqf = np.asarray(bq, np.float32).reshape(N_HEADS * HEAD_DIM)
    bkf = np.asarray(bk, np.float32).reshape(N_KV * HEAD_DIM)
    bvf = np.asarray(bv, np.float32).reshape(N_KV * HEAD_DIM)

    in_maps = []
    for i in range(N_CORES):
        # concat q|k|v slices: [5120, 896]
        w_cat = np.concatenate(
            [
                wq[:, i * QD : (i + 1) * QD],
                wk[:, i * HEAD_DIM : (i + 1) * HEAD_DIM],
                wv[:, i * HEAD_DIM : (i + 1) * HEAD_DIM],
            ],
            axis=1,
        )
        b_cat = np.concatenate(
            [
                bqf[i * QD : (i + 1) * QD],
                bkf[i * HEAD_DIM : (i + 1) * HEAD_DIM],
                bvf[i * HEAD_DIM : (i + 1) * HEAD_DIM],
            ]
        ).reshape(1, WKV)
        wo_i = _part_major(np.asarray(wo, np.float32)[i * QD : (i + 1) * QD, :])
        kt_i = np.ascontiguousarray(
            np.asarray(cache_k, np.float32)[:, :, i, :].transpose(0, 2, 1)
        )  # [8, 128, 4096]
        v_raw = np.asarray(cache_v, np.float32)[:, :, i, :].reshape(
            NB, NCH, 128, HEAD_DIM
        )
        v_ext = np.ones((NB, NCH, 128, VE), np.float32)
        v_ext[..., :HEAD_DIM] = v_raw
        v_i = np.ascontiguousarray(
            v_ext.transpose(0, 2, 1, 3).reshape(NB, 128, NCH * VE)
        )  # [8, 128, 4128]
        in_maps.append(
            dict(
                xt=xt,
                wqkv=_part_major(w_cat),
                bqkv=np.ascontiguousarray(b_cat),
                arope=arope,
                ones8=np.ones((1, NB), np.float32),
                kt=kt_i,
                v=v_i,
                wo=wo_i,
                bo8=bo8,
            )
        )
    return in_maps


def build_graph():
    import concourse.mybir as mybir
    from concourse import bacc
    from concourse.masks import make_identity
    from concourse.tile import TileContext

    f32 = mybir.dt.float32
    f32r = mybir.dt.float32r
    bf16 = mybir.dt.bfloat16
    nc = bacc.Bacc(num_devices=N_CORES, name="attn_decode_tp8")

    xt_p = nc.declare_dram_parameter("xt", [128, KD * NB], f32r, isOutput=False)
    wqkv_p = nc.declare_dram_parameter("wqkv", [128, KD * WKV], f32r, isOutput=False)
    bqkv_p = nc.declare_dram_parameter("bqkv", [1, WKV], f32r, isOutput=False)
    arope_p = nc.declare_dram_parameter("arope", [128, 128], f32, isOutput=False)
    ones8_p = nc.declare_dram_parameter("ones8", [1, NB], f32r, isOutput=False)
    kt_p = nc.declare_dram_parameter("kt", [NB, 128, MAX_SEQ], f32, isOutput=False)
    v_p = nc.declare_dram_parameter("v", [NB, 128, NCH * VE], f32, isOutput=False)
    wo_p = nc.declare_dram_parameter("wo", [128, HPC * DIM], f32r, isOutput=False)
    bo8_p = nc.declare_dram_parameter("bo8", [1, DIM], f32r, isOutput=False)
    out_p = nc.declare_dram_parameter("out", [NB, DIM], f32, isOutput=True)

    Exp = mybir.ActivationFunctionType.Exp

    with TileContext(nc, num_cores=N_CORES) as tc:
        with (
            tc.tile_pool(name="const", bufs=1) as constp,
            tc.tile_pool(name="persist", bufs=1) as pers,
            tc.tile_pool(name="dram", bufs=1, space="DRAM") as dramp,
        ):
            identity = constp.tile([128, 128], f32)
            make_identity(nc, identity)
            ones8 = constp.tile([1, NB], f32r)
            nc.sync.dma_start(ones8[:], ones8_p[:])
            arope_sb = constp.tile([128, 128], f32)
            nc.sync.dma_start(arope_sb[:], arope_p[:])
            bqkv_sb = constp.tile([1, WKV], f32r)
            nc.sync.dma_start(bqkv_sb[:], bqkv_p[:])

            qr_sb = pers.tile([128, NB * HPC], bf16)  # roped q^T, cols b*5+h
            knT_sb = pers.tile([128, NB], bf16)  # roped new-k^T, cols b
            xv_sb = pers.tile([NB, HEAD_DIM], bf16)  # new v rows
            attnT_sb = pers.tile([128, HPC * NB], f32r)  # cols h*8+b

            # ---------------- Phase A: projections + rope ----------------
            with (
                tc.tile_pool(name="pha", bufs=1) as pha,
                tc.tile_pool(name="wqp", bufs=2) as wqp,
                tc.tile_pool(name="phaps", bufs=1, space="PSUM") as phaps,
            ):
                xt_sb = pha.tile([128, KD * NB], f32r)
                nc.sync.dma_start(xt_sb[:], xt_p[:])

                qkv_ps = phaps.tile([NB, WKV], f32)
                nc.tensor.matmul(
                    qkv_ps[:, 0:512], ones8[:], bqkv_sb[:, 0:512], start=True, stop=False
                )
                nc.tensor.matmul(
                    qkv_ps[:, 512:WKV],
                    ones8[:],
                    bqkv_sb[:, 512:WKV],
                    start=True,
                    stop=False,
                )
                GK = 10  # k-chunks per wqkv tile
                for g in range(KD // GK):
                    wq_sb = wqp.tile([128, GK * WKV], f32r, tag="wq")
                    nc.sync.dma_start(
                        wq_sb[:], wqkv_p[:, g * GK * WKV : (g + 1) * GK * WKV]
                    )
                    for o in range(GK):
                        k = g * GK + o
                        lhsT = xt_sb[:, k * NB : (k + 1) * NB]
                        last = k == KD - 1
                        nc.tensor.matmul(
                            qkv_ps[:, 0:512],
                            lhsT,
                            wq_sb[:, o * WKV : o * WKV + 512],
                            start=False,
                            stop=last,
                        )
                        nc.tensor.matmul(
                            qkv_ps[:, 512:WKV],
                            lhsT,
                            wq_sb[:, o * WKV + 512 : (o + 1) * WKV],
                            start=False,
                            stop=last,
                        )

                # copy q|k parts to f32 for transposes; v part to bf16
                qk_sb = pha.tile([NB, QD + HEAD_DIM], f32)
                nc.vector.tensor_copy(qk_sb[:], qkv_ps[:, 0 : QD + HEAD_DIM])
                nc.vector.tensor_copy(xv_sb[:], qkv_ps[:, QD + HEAD_DIM : WKV])

                qkT_sb = pha.tile([128, NB * HPC + NB], f32)
                qkT_q3 = qkT_sb[:, 0 : NB * HPC].rearrange("p (b h) -> p b h", h=HPC)
                for h in range(HPC):
                    tq_ps = phaps.tile([128, NB], f32, tag="tq", bufs=2)
                    nc.tensor.transpose(
                        tq_ps[:],
                        qk_sb[:, h * HEAD_DIM : (h + 1) * HEAD_DIM],
                        identity[0:NB, 0:NB],
                    )
                    nc.vector.tensor_copy(qkT_q3[:, :, h], tq_ps[:])
                tk_ps = phaps.tile([128, NB], f32, tag="tq", bufs=2)
                nc.tensor.transpose(
                    tk_ps[:], qk_sb[:, QD : QD + HEAD_DIM], identity[0:NB, 0:NB]
                )
                nc.vector.tensor_copy(qkT_sb[:, NB * HPC : NB * HPC + NB], tk_ps[:])

                qkr_ps = phaps.tile([128, NB * HPC + NB], f32)
                nc.tensor.matmul(
                    qkr_ps[:], arope_sb[:], qkT_sb[:], start=True, stop=True
                )
                nc.vector.tensor_copy(qr_sb[:], qkr_ps[:, 0 : NB * HPC])
                nc.vector.tensor_copy(
                    knT_sb[:], qkr_ps[:, NB * HPC : NB * HPC + NB]
                )

            # ---------------- Phase B: attention per batch ----------------
            wo_tiles = []
            with tc.tile_pool(name="wop", bufs=HPC) as wop:
                with (
                    tc.tile_pool(name="ktp", bufs=2) as ktp,
                    tc.tile_pool(name="vvp", bufs=2) as vvp,
                    tc.tile_pool(name="psm", bufs=2) as psm,
                    tc.tile_pool(name="aps", bufs=2, space="PSUM") as aps,
                ):
                    attnT_3 = attnT_sb.rearrange("p (h b) -> p h b", b=NB)
                    for b in range(NB):
                        ktb = ktp.tile([128, MAX_SEQ], bf16, tag="ktb")
                        nc.gpsimd.dma_start(ktb[:], kt_p[b])  # f32 -> bf16 cast
                        vb = vvp.tile([128, NCH * VE], bf16, tag="vb")
                        nc.gpsimd.dma_start(vb[:], v_p[b])  # f32 -> bf16 cast
                        # patch in the new token's k (col 4095) and v (row 127
                        # of last chunk)
                        nc.vector.tensor_copy(
                            ktb[:, MAX_SEQ - 1 : MAX_SEQ], knT_sb[:, b : b + 1]
                        )
                        nc.sync.dma_start(
                            vb[127:128, (NCH - 1) * VE : (NCH - 1) * VE + HEAD_DIM],
                            xv_sb[b : b + 1, :],
                        )

                        s_ps = aps.tile([128, NCH * HPC], f32, tag="s")
                        for c in range(NCH):
                            nc.tensor.matmul(
                                s_ps[:, c * HPC : (c + 1) * HPC],
                                ktb[:, c * 128 : (c + 1) * 128],
                                qr_sb[:, b * HPC : (b + 1) * HPC],
                                start=True,
                                stop=True,
                            )
                        p_sb = psm.tile([128, NCH * HPC], bf16, tag="p")
                        nc.scalar.activation(p_sb[:], s_ps[:], Exp, scale=SCALE)

                        o_ps = aps.tile([HPC, VE], f32, tag="o")
                        for c in range(NCH):
                            nc.tensor.matmul(
                                o_ps[:],
                                p_sb[:, c * HPC : (c + 1) * HPC],
                                vb[:, c * VE : (c + 1) * VE],
                                start=(c == 0),
                                stop=(c == NCH - 1),
                            )
                        r_sb = psm.tile([HPC, 1], f32, tag="r")
                        nc.vector.reciprocal(r_sb[:], o_ps[:, HEAD_DIM : HEAD_DIM + 1])
                        attn_b = psm.tile([HPC, HEAD_DIM], f32, tag="attn_b")
                        nc.vector.tensor_scalar_mul(
                            attn_b[:], o_ps[:, 0:HEAD_DIM], r_sb[:]
                        )
                        ta_ps = aps.tile([128, HPC], f32, tag="ta")
                        nc.tensor.transpose(
                            ta_ps[:], attn_b[:], identity[0:HPC, 0:HPC]
                        )
                        nc.vector.tensor_copy(attnT_3[:, :, b], ta_ps[:])

                        if b >= NB - HPC:  # stream wo tiles in late
                            h = b - (NB - HPC)
                            wo_sb = wop.tile([128, DIM], f32r, tag="wo")
                            nc.sync.dma_start(
                                wo_sb[:], wo_p[:, h * DIM : (h + 1) * DIM]
                            )
                            wo_tiles.append(wo_sb)

                # ---------------- Phase C: o_proj + AllReduce ----------------
                with (
                    tc.tile_pool(name="opsp", bufs=1, space="PSUM") as opsp,
                    tc.tile_pool(name="oop", bufs=1) as oop,
                ):
                    bo8_sb = oop.tile([1, DIM], f32r)
                    nc.sync.dma_start(bo8_sb[:], bo8_p[:])
                    oo_sb = oop.tile([NB, DIM], f32)
                    HN = DIM // 2  # 2560 per round
                    for r in range(2):
                        op_ps = opsp.tile([NB, HN], f32, tag="op")
                        for j in range(HN // 512):
                            nc.tensor.matmul(
                                op_ps[:, j * 512 : (j + 1) * 512],
                                ones8[:],
                                bo8_sb[:, r * HN + j * 512 : r * HN + (j + 1) * 512],
                                start=True,
                                stop=False,
                            )
                        for h in range(HPC):
                            lhsT = attnT_sb[:, h * NB : (h + 1) * NB]
                            for j in range(HN // 512):
                                nc.tensor.matmul(
                                    op_ps[:, j * 512 : (j + 1) * 512],
                                    lhsT,
                                    wo_tiles[h][
                                        :, r * HN + j * 512 : r * HN + (j + 1) * 512
                                    ],
                                    start=False,
                                    stop=(h == HPC - 1),
                                )
                        nc.vector.tensor_copy(oo_sb[:, r * HN : (r + 1) * HN], op_ps[:])

                    cc_in = dramp.tile([NB, DIM], f32)
                    cc_out = dramp.tile([NB, DIM], f32)
                    nc.sync.dma_start(cc_in[:], oo_sb[:])
                    nc.gpsimd.collective_compute(
                        "AllReduce",
                        mybir.AluOpType.add,
                        replica_groups=[list(range(N_CORES))],
                        ins=[cc_in.opt()],
                        outs=[cc_out.opt()],
                    )
                    nc.gpsimd.dma_start(out_p[:], cc_out[:])

    nc.finalize()
    return nc


def _execute(inputs: dict, trace: bool = False):
    from concourse.bass_utils import run_bass_kernel_spmd

    start_pos = int(np.asarray(inputs["start_pos"]))
    assert start_pos + 1 == MAX_SEQ, f"kernel hardcoded for klen=4096, got {start_pos}"

    in_maps = prepare_inputs(
        inputs["x"],
        inputs["freqs_cis"],
        inputs["cache_k"],
        inputs["cache_v"],
        inputs["wq"],
        inputs["bq"],
        inputs["wk"],
        inputs["bk"],
        inputs["wv"],
        inputs["bv"],
        inputs["wo"],
        inputs["bo"],
    )
    nc = build_graph()
    res = run_bass_kernel_spmd(
        nc, in_maps, core_ids=list(range(N_CORES)), trace=trace
    )
    out = res.results[0]["out"].reshape(NB, 1, DIM).astype(np.float32)
    return out, res


def kernel(**inputs) -> np.ndarray:
    return _execute(inputs, trace=False)[0]


# revision 14
# speedup vs baseline: 1.1063x; 1.1063x over previous
"""Trainium2 Bass kernel for GQA decode attention (nn_Attention_37890201485423).

Tensor-parallel over KV heads: 8 cores x (1 KV head + 5 query heads each).
o_proj is row-sharded with a final AllReduce.

Layout strategy (all host-side prep is numpy; all module math runs on device):
  - K cache shard is shipped pre-transposed per batch as [d=128, t=4096] so the
    scores matmul can use K^T chunks as the stationary operand directly.
  - V cache shard is shipped as 128-token chunks with a ones column appended
    ([t, 129]); the AV matmul then produces numerator and softmax denominator
    in one accumulation group.
  - RoPE is applied on device as a single matmul with a 128x128 block-diagonal
    rotation matrix built from freqs_cis (standard precomputed rotary table).
  - All weights are pre-rearranged partition-major so each DMA is contiguous.

Dtype strategy: HBM traffic stays f32 (the memory workload), but all matmul
operands are cast f32->bf16 during the SWDGE DMA (PE fp32 matmuls are 2-4x
slower than bf16). All matmul accumulation is fp32 in PSUM; softmax
normalization is fp32. Measured end-to-end relative error ~4e-3.
"""

import sys

import numpy as np

if "/opt/trn_rl_repo" not in sys.path:
    sys.path.insert(0, "/opt/trn_rl_repo")


def _install_ntff_hook():
    """The container's antenv stub lacks axon_hooks; recreate it so
    run_bass_kernel_spmd(trace=True) can capture NTFF profiles via the
    libaxon ctypes path (mirrors trn_agent_boot.trn_boot)."""
    import types

    if "antenv.axon_hooks" in sys.modules:
        return
    mod = types.ModuleType("antenv.axon_hooks")
    mod._hook = None

    def set_axon_ntff_profile_hook(h):
        mod._hook = h

    def get_axon_ntff_profile_hook():
        return mod._hook

    mod.set_axon_ntff_profile_hook = set_axon_ntff_profile_hook
    mod.get_axon_ntff_profile_hook = get_axon_ntff_profile_hook
    sys.modules["antenv.axon_hooks"] = mod
    try:
        import antenv

        antenv.axon_hooks = mod
    except ImportError:
        pass
    try:
        boot_dir = "/root/.axon_site/trn_agent_boot"
        if boot_dir not in sys.path:
            sys.path.insert(0, boot_dir)
        import trn_boot

        hook = trn_boot._ntff_profile_via_ctypes("/opt/axon/libaxon_pjrt.so")
        if hook is not None:
            mod._hook = hook
    except Exception:
        pass


_install_ntff_hook()

DIM, N_HEADS, N_KV, HEAD_DIM = 5120, 40, 8, 128
MAX_BS, MAX_SEQ = 8, 4096
NB = 8  # batch
N_CORES = 8
N_REP = N_HEADS // N_KV  # 5 query heads per kv head
HPC = N_REP  # heads per core
QD = HPC * HEAD_DIM  # 640, per-core q/o width
WKV = QD + 2 * HEAD_DIM  # 896: concat q|k|v projection width per core
KD = 40  # contraction chunks for DIM
NCH = 32  # 4096 / 128 token chunks
VE = HEAD_DIM + 1  # 129, v chunk width with ones column
SCALE = 1.0 / float(np.sqrt(HEAD_DIM))


def _build_rope_matrix(freqs_cis: np.ndarray) -> np.ndarray:
    """lhsT for the rope matmul: out = lhsT.T @ rhs applies the rotation A.

    A[2i,2i]=cos_i, A[2i,2i+1]=-sin_i, A[2i+1,2i]=sin_i, A[2i+1,2i+1]=cos_i
    (matches reference _apply_rope with interleaved even/odd pairs).
    """
    cos = np.asarray(freqs_cis, np.float32)[0, :, 0]
    sin = np.asarray(freqs_cis, np.float32)[0, :, 1]
    A = np.zeros((HEAD_DIM, HEAD_DIM), np.float32)
    idx = np.arange(HEAD_DIM // 2)
    A[2 * idx, 2 * idx] = cos
    A[2 * idx, 2 * idx + 1] = -sin
    A[2 * idx + 1, 2 * idx] = sin
    A[2 * idx + 1, 2 * idx + 1] = cos
    return np.ascontiguousarray(A.T)


def _part_major(w: np.ndarray) -> np.ndarray:
    """[K*128, N] -> [128, K*N] with chunk k in columns k*N:(k+1)*N."""
    k = w.shape[0] // 128
    return np.ascontiguousarray(
        w.reshape(k, 128, w.shape[1]).transpose(1, 0, 2).reshape(128, -1)
    )


def prepare_inputs(x, freqs_cis, cache_k, cache_v, wq, bq, wk, bk, wv, bv, wo, bo):
    """Returns per-core in_maps (list of dicts of numpy arrays)."""
    x = np.asarray(x, np.float32).reshape(NB, DIM)
    arope = _build_rope_matrix(freqs_cis)

    # xt: stationary operand layout for the projections.
    # xt[:, k*8:(k+1)*8][p, c] = x[c, k*128+p]
    xs = x.reshape(NB, KD, 128)
    xt = np.ascontiguousarray(xs.transpose(2, 1, 0).reshape(128, KD * NB))

    bo8 = (np.asarray(bo, np.float32) / N_CORES).reshape(1, DIM)
    wq, wk, wv = (np.asarray(a, np.float32) for a in (wq, wk, wv))
    bqf = np.asarray(bq, np.float32).reshape(N_HEADS * HEAD_DIM)
    bkf = np.asarray(bk, np.float32).reshape(N_KV * HEAD_DIM)
    bvf = np.asarray(bv, np.float32).reshape(N_KV * HEAD_DIM)

    in_maps = []
    for i in range(N_CORES):
        # concat q|k|v slices: [5120, 896]
        w_cat = np.concatenate(
            [
                wq[:, i * QD : (i + 1) * QD],
                wk[:, i * HEAD_DIM : (i + 1) * HEAD_DIM],
                wv[:, i * HEAD_DIM : (i + 1) * HEAD_DIM],
            ],
            axis=1,
        )
        b_cat = np.concatenate(
            [
                bqf[i * QD : (i + 1) * QD],
                bkf[i * HEAD_DIM : (i + 1) * HEAD_DIM],
                bvf[i * HEAD_DIM : (i + 1) * HEAD_DIM],
            ]
        ).reshape(1, WKV)
        wo_i = _part_major(np.asarray(wo, np.float32)[i * QD : (i + 1) * QD, :])
        kt_i = np.ascontiguousarray(
            np.asarray(cache_k, np.float32)[:, :, i, :].transpose(0, 2, 1)
        )  # [8, 128, 4096]
        v_raw = np.asarray(cache_v, np.float32)[:, :, i, :].reshape(
            NB, NCH, 128, HEAD_DIM
        )
        v_ext = np.ones((NB, NCH, 128, VE), np.float32)
        v_ext[..., :HEAD_DIM] = v_raw
        v_i = np.ascontiguousarray(
            v_ext.transpose(0, 2, 1, 3).reshape(NB, 128, NCH * VE)
        )  # [8, 128, 4128]
        in_maps.append(
            dict(
                xt=xt,
                wqkv=_part_major(w_cat),
                bqkv=np.ascontiguousarray(b_cat),
                arope=arope,
                kt=kt_i,
                v=v_i,
                wo=wo_i,
                bo8=bo8,
            )
        )
    return in_maps


def build_graph():
    import concourse.mybir as mybir
    from concourse import bacc
    from concourse.masks import make_identity
    from concourse.tile import TileContext

    f32 = mybir.dt.float32
    bf16 = mybir.dt.bfloat16
    nc = bacc.Bacc(num_devices=N_CORES, name="attn_decode_tp8")

    xt_p = nc.declare_dram_parameter("xt", [128, KD * NB], f32, isOutput=False)
    wqkv_p = nc.declare_dram_parameter("wqkv", [128, KD * WKV], f32, isOutput=False)
    bqkv_p = nc.declare_dram_parameter("bqkv", [1, WKV], f32, isOutput=False)
    arope_p = nc.declare_dram_parameter("arope", [128, 128], f32, isOutput=False)
    kt_p = nc.declare_dram_parameter("kt", [NB, 128, MAX_SEQ], f32, isOutput=False)
    v_p = nc.declare_dram_parameter("v", [NB, 128, NCH * VE], f32, isOutput=False)
    wo_p = nc.declare_dram_parameter("wo", [128, HPC * DIM], f32, isOutput=False)
    bo8_p = nc.declare_dram_parameter("bo8", [1, DIM], f32, isOutput=False)
    out_p = nc.declare_dram_parameter("out", [NB, DIM], f32, isOutput=True)

    Exp = mybir.ActivationFunctionType.Exp

    with TileContext(nc, num_cores=N_CORES) as tc:
        with (
            tc.tile_pool(name="const", bufs=1) as constp,
            tc.tile_pool(name="persist", bufs=1) as pers,
            tc.tile_pool(name="dram", bufs=1, space="DRAM") as dramp,
            tc.tile_pool(name="ktp", bufs=2) as ktp,
            tc.tile_pool(name="vvp", bufs=2) as vvp,
            tc.tile_pool(name="wop", bufs=HPC) as wop,
        ):
            identity = constp.tile([128, 128], f32)
            make_identity(nc, identity)
            ones8 = constp.tile([1, NB], bf16)
            nc.gpsimd.memset(ones8[:], 1.0)
            arope_sb = constp.tile([128, 128], f32)
            nc.sync.dma_start(arope_sb[:], arope_p[:])
            bqkv_sb = constp.tile([1, WKV], bf16)
            nc.gpsimd.dma_start(bqkv_sb[:], bqkv_p[:])  # f32 -> bf16

            qr_sb = pers.tile([128, NB * HPC], bf16)  # roped q^T, cols b*5+h
            knT_sb = pers.tile([128, NB], bf16)  # roped new-k^T, cols b
            xv_sb = pers.tile([NB, HEAD_DIM], bf16)  # new v rows
            attnT_sb = pers.tile([128, HPC * NB], bf16)  # cols h*8+b

            kt_tiles, v_tiles = {}, {}

            def load_kv(b):
                ktb = ktp.tile([128, MAX_SEQ], bf16, tag="ktb", name=f"ktb{b}")
                nc.gpsimd.dma_start(ktb[:], kt_p[b])  # f32 -> bf16 cast
                vb = vvp.tile([128, NCH * VE], bf16, tag="vb", name=f"vb{b}")
                nc.gpsimd.dma_start(vb[:], v_p[b])  # f32 -> bf16 cast
                kt_tiles[b], v_tiles[b] = ktb, vb

            # ---------------- Phase A: projections + rope ----------------
            with (
                tc.tile_pool(name="pha", bufs=1) as pha,
                tc.tile_pool(name="wqp", bufs=4) as wqp,
                tc.tile_pool(name="phaps", bufs=1, space="PSUM") as phaps,
            ):
                xt_sb = pha.tile([128, KD * NB], bf16)
                nc.gpsimd.dma_start(xt_sb[:], xt_p[:])  # f32 -> bf16

                GK = 10  # k-chunks per wqkv tile
                wq_tiles = []
                for g in range(KD // GK):
                    wq_sb = wqp.tile(
                        [128, GK * WKV], bf16, tag="wq", name=f"wq{g}"
                    )
                    nc.gpsimd.dma_start(
                        wq_sb[:], wqkv_p[:, g * GK * WKV : (g + 1) * GK * WKV]
                    )
                    wq_tiles.append(wq_sb)
                # start the first KV cache loads right behind the weights
                load_kv(0)
                load_kv(1)

                qkv_ps = phaps.tile([NB, WKV], f32)
                nc.tensor.matmul(
                    qkv_ps[:, 0:512],
                    ones8[:],
                    bqkv_sb[:, 0:512],
                    start=True,
                    stop=False,
                )
                nc.tensor.matmul(
                    qkv_ps[:, 512:WKV],
                    ones8[:],
                    bqkv_sb[:, 512:WKV],
                    start=True,
                    stop=False,
                )
                for g in range(KD // GK):
                    wq_sb = wq_tiles[g]
                    for o in range(GK):
                        k = g * GK + o
                        lhsT = xt_sb[:, k * NB : (k + 1) * NB]
                        last = k == KD - 1
                        nc.tensor.matmul(
                            qkv_ps[:, 0:512],
                            lhsT,
                            wq_sb[:, o * WKV : o * WKV + 512],
                            start=False,
                            stop=last,
                        )
                        nc.tensor.matmul(
                            qkv_ps[:, 512:WKV],
                            lhsT,
                            wq_sb[:, o * WKV + 512 : (o + 1) * WKV],
                            start=False,
                            stop=last,
                        )

                # copy q|k parts to f32 for transposes; v part to bf16
                qk_sb = pha.tile([NB, QD + HEAD_DIM], f32)
                nc.vector.tensor_copy(qk_sb[:], qkv_ps[:, 0 : QD + HEAD_DIM])
                nc.vector.tensor_copy(xv_sb[:], qkv_ps[:, QD + HEAD_DIM : WKV])

                qkT_sb = pha.tile([128, NB * HPC + NB], f32)
                qkT_q3 = qkT_sb[:, 0 : NB * HPC].rearrange("p (b h) -> p b h", h=HPC)
                for h in range(HPC):
                    tq_ps = phaps.tile([128, NB], f32, tag="tq", bufs=2)
                    nc.tensor.transpose(
                        tq_ps[:],
                        qk_sb[:, h * HEAD_DIM : (h + 1) * HEAD_DIM],
                        identity[0:NB, 0:NB],
                    )
                    nc.vector.tensor_copy(qkT_q3[:, :, h], tq_ps[:])
                tk_ps = phaps.tile([128, NB], f32, tag="tq", bufs=2)
                nc.tensor.transpose(
                    tk_ps[:], qk_sb[:, QD : QD + HEAD_DIM], identity[0:NB, 0:NB]
                )
                nc.vector.tensor_copy(qkT_sb[:, NB * HPC : NB * HPC + NB], tk_ps[:])

                qkr_ps = phaps.tile([128, NB * HPC + NB], f32)
                nc.tensor.matmul(
                    qkr_ps[:], arope_sb[:], qkT_sb[:], start=True, stop=True
                )
                nc.vector.tensor_copy(qr_sb[:], qkr_ps[:, 0 : NB * HPC])
                nc.vector.tensor_copy(
                    knT_sb[:], qkr_ps[:, NB * HPC : NB * HPC + NB]
                )

            # ---------------- Phase B: attention per batch ----------------
            wo_tiles = []
            with (
                tc.tile_pool(name="psm", bufs=2) as psm,
                tc.tile_pool(name="aps", bufs=2, space="PSUM") as aps,
            ):
                attnT_3 = attnT_sb.rearrange("p (h b) -> p h b", b=NB)
                for b in range(NB):
                    if b not in kt_tiles:
                        load_kv(b)
                    ktb, vb = kt_tiles[b], v_tiles[b]
                    # patch in the new token's k (col 4095) and v (row 127 of
                    # last chunk)
                    nc.vector.tensor_copy(
                        ktb[:, MAX_SEQ - 1 : MAX_SEQ], knT_sb[:, b : b + 1]
                    )
                    nc.sync.dma_start(
                        vb[127:128, (NCH - 1) * VE : (NCH - 1) * VE + HEAD_DIM],
                        xv_sb[b : b + 1, :],
                    )

                    s_ps = aps.tile([128, NCH * HPC], f32, tag="s")
                    for c in range(NCH):
                        nc.tensor.matmul(
                            s_ps[:, c * HPC : (c + 1) * HPC],
                            ktb[:, c * 128 : (c + 1) * 128],
                            qr_sb[:, b * HPC : (b + 1) * HPC],
                            start=True,
                            stop=True,
                        )
                    p_sb = psm.tile([128, NCH * HPC], bf16, tag="p")
                    nc.scalar.activation(p_sb[:], s_ps[:], Exp, scale=SCALE)

                    o_ps = aps.tile([HPC, VE], f32, tag="o")
                    for c in range(NCH):
                        nc.tensor.matmul(
                            o_ps[:],
                            p_sb[:, c * HPC : (c + 1) * HPC],
                            vb[:, c * VE : (c + 1) * VE],
                            start=(c == 0),
                            stop=(c == NCH - 1),
                        )
                    r_sb = psm.tile([HPC, 1], f32, tag="r")
                    nc.vector.reciprocal(r_sb[:], o_ps[:, HEAD_DIM : HEAD_DIM + 1])
                    attn_b = psm.tile([HPC, HEAD_DIM], f32, tag="attn_b")
                    nc.vector.tensor_scalar_mul(
                        attn_b[:], o_ps[:, 0:HEAD_DIM], r_sb[:]
                    )
                    ta_ps = aps.tile([128, HPC], f32, tag="ta")
                    nc.tensor.transpose(ta_ps[:], attn_b[:], identity[0:HPC, 0:HPC])
                    nc.vector.tensor_copy(attnT_3[:, :, b], ta_ps[:])

                    if b >= NB - HPC:  # stream wo tiles in late
                        h = b - (NB - HPC)
                        wo_sb = wop.tile([128, DIM], bf16, tag="wo", name=f"wo{h}")
                        nc.gpsimd.dma_start(
                            wo_sb[:], wo_p[:, h * DIM : (h + 1) * DIM]
                        )
                        wo_tiles.append(wo_sb)

            # ---------------- Phase C: o_proj + AllReduce ----------------
            with (
                tc.tile_pool(name="opsp", bufs=1, space="PSUM") as opsp,
                tc.tile_pool(name="oop", bufs=1) as oop,
            ):
                bo8_sb = oop.tile([1, DIM], bf16)
                nc.gpsimd.dma_start(bo8_sb[:], bo8_p[:])  # f32 -> bf16
                oo_sb = oop.tile([NB, DIM], f32)
                HN = DIM // 2  # 2560 per round
                for r in range(2):
                    op_ps = opsp.tile([NB, HN], f32, tag="op")
                    for j in range(HN // 512):
                        nc.tensor.matmul(
                            op_ps[:, j * 512 : (j + 1) * 512],
                            ones8[:],
                            bo8_sb[:, r * HN + j * 512 : r * HN + (j + 1) * 512],
                            start=True,
                            stop=False,
                        )
                    for h in range(HPC):
                        lhsT = attnT_sb[:, h * NB : (h + 1) * NB]
                        for j in range(HN // 512):
                            nc.tensor.matmul(
                                op_ps[:, j * 512 : (j + 1) * 512],
                                lhsT,
                                wo_tiles[h][
                                    :, r * HN + j * 512 : r * HN + (j + 1) * 512
                                ],
                                start=False,
                                stop=(h == HPC - 1),
                            )
                    nc.vector.tensor_copy(oo_sb[:, r * HN : (r + 1) * HN], op_ps[:])

                cc_in = dramp.tile([NB, DIM], f32)
                cc_out = dramp.tile([NB, DIM], f32)
                nc.sync.dma_start(cc_in[:], oo_sb[:])
                nc.gpsimd.collective_compute(
                    "AllReduce",
                    mybir.AluOpType.add,
                    replica_groups=[list(range(N_CORES))],
                    ins=[cc_in.opt()],
                    outs=[cc_out.opt()],
                )
                nc.gpsimd.dma_start(out_p[:], cc_out[:])

    nc.finalize()
    return nc


def _execute(inputs: dict, trace: bool = False):
    from concourse.bass_utils import run_bass_kernel_spmd

    start_pos = int(np.asarray(inputs["start_pos"]))
    assert start_pos + 1 == MAX_SEQ, f"kernel hardcoded for klen=4096, got {start_pos}"

    in_maps = prepare_inputs(
        inputs["x"],
        inputs["freqs_cis"],
        inputs["cache_k"],
        inputs["cache_v"],
        inputs["wq"],
        inputs["bq"],
        inputs["wk"],
        inputs["bk"],
        inputs["wv"],
        inputs["bv"],
        inputs["wo"],
        inputs["bo"],
    )
    nc = build_graph()
    res = run_bass_kernel_spmd(
        nc, in_maps, core_ids=list(range(N_CORES)), trace=trace
    )
    out = res.results[0]["out"].reshape(NB, 1, DIM).astype(np.float32)
    return out, res


def kernel(**inputs) -> np.ndarray:
    return _execute(inputs, trace=False)[0]
